# revision 5
# baseline (speedup 1.0000x reference)
"""ConvGRU Trainium2 kernel (v2).

video [B=2, T=16, C=128, H=64, W=64] f32; 1x1-conv GRU over T.
Sharding: data-parallel over (B x H/16) -> 8 cores, each core owns
P = 16*64 = 1024 pixels for all T; weights replicated.

Layout per core: channels on partitions, pixels on the free dim.
G=2 pixel groups of PG=512 form two independent recurrence chains.

Per step (h' = (1-z)h + z*tanh(c)) computed as:
    z   = sigmoid(Z + bz)            ACT [1024]   (both groups fused)
    r_g = sigmoid(R_g + br)          ACT [512]    (chain-critical, early)
    rh_g = r_g * h_g                 DVE [512]
    c_g = tanh(C_g + bh)             ACT [512]
    q   = (z - 1) * h                DVE STT [1024] (off-chain)
    v_g = z_g * c_g                  DVE [512]
    h'_g = v_g - q_g                 DVE [512]    (= z*c + (1-z)*h)
where Z/R/C are PSUM accumulations: opener (W?x @ x_{t}, issued during
step t-1) + closer (W?h @ h or rh).

Instruction-count discipline (HW-measured: mm[512]=379ns=0.417N+165,
ACT=0.833N+260, DVE TT fp16=0.521N+160): PE 12, ACT 5, DVE 8 per step.
t=0 runs a reduced step (h=0): no closes / sigr / rh / q.

Numerics: fp16 matmul inputs/gates/state, fp32 PSUM accum + fp32 bias.
"""

import os
import sys

import numpy as np

B, T, C, H, W = 2, 16, 128, 64, 64
NCORES = 8
HQ = H // 4          # 16 rows of H per core (4 H-slices x 2 batches = 8 cores)
P = HQ * W           # 1024 pixels per core
G = 2                # pixel groups per step (independent recurrence chains)
PG = P // G          # 512 pixels per group

_PROG = None


def _ensure_paths():
    for p in ("/opt/trn_rl_repo",):
        if p not in sys.path and os.path.isdir(p):
            sys.path.append(p)


def _build():
    _ensure_paths()
    import concourse.bacc as bacc
    import concourse.tile as tile
    from concourse import mybir

    f32 = mybir.dt.float32
    f16 = mybir.dt.float16
    AF = mybir.ActivationFunctionType
    ALU = mybir.AluOpType

    nc = bacc.Bacc(
        "TRN2", target_bir_lowering=False, debug=False, num_devices=NCORES
    )
    x_dram = nc.dram_tensor("x_seq", [T, C, P], f16, kind="ExternalInput")
    w_dram = nc.dram_tensor("wmats", [C, 6 * C], f16, kind="ExternalInput")
    b_dram = nc.dram_tensor("biases", [C, 3], f32, kind="ExternalInput")
    o_dram = nc.dram_tensor("out_seq", [T, C, P], f16, kind="ExternalOutput")

    x_ap = x_dram.ap()
    w_ap = w_dram.ap()
    b_ap = b_dram.ap()
    o_ap = o_dram.ap()

    WZX, WZH, WRX, WRH, WHX, WHH = range(6)

    with tile.TileContext(nc) as tc:
        with (
            tc.tile_pool(name="consts", bufs=1) as consts,
            tc.tile_pool(name="xin", bufs=3) as xpool,
            tc.tile_pool(name="state", bufs=2) as spool,
            tc.tile_pool(name="work", bufs=2) as wk,
            tc.tile_pool(name="ps", bufs=1, space="PSUM") as ps,
        ):
            # garbage tile for warmup matmuls: no DMA dependency, so the
            # PE clock ramp + ACT table load start at kernel begin and
            # fully overlap the input DMAs
            junk = consts.tile([C, PG], f16)
            nc.gpsimd.memset(junk[:], 0.0)

            wt = consts.tile([C, 6 * C], f16)
            nc.sync.dma_start(wt[:], w_ap[:])
            bt = consts.tile([C, 3], f32)
            nc.gpsimd.dma_start(bt[:], b_ap[:])

            def wslice(i):
                return wt[:, i * C : (i + 1) * C]

            # PSUM: Z [C,1024] (2 banks), R_g [C,512] x2 (2), C_g x2 bufs=2 (4)
            zp = ps.tile([C, P], f32, tag="Z", bufs=1)

            def zs(g):
                return zp[:, g * PG : (g + 1) * PG]

            for i in range(6):
                nc.tensor.matmul(
                    zp[:, :PG], junk[:, :C], junk[:], start=True, stop=True
                )
            wtmp = wk.tile([C, PG], f16, tag="c16_0")
            nc.scalar.activation(
                wtmp[:], zp[:, :PG], AF.Sigmoid, bias=0.0
            )

            def load_x(t):
                xt = xpool.tile([C, P], f16, tag="x")
                nc.sync.dma_start(xt[:], x_ap[t])
                return xt

            x_t = load_x(0)
            x_next = load_x(1)

            # -- t = 0 (h == 0): z/c only, h1 = z*c --
            cp_t = [None, None]
            for g in range(G):
                xs = x_t[:, g * PG : (g + 1) * PG]
                nc.tensor.matmul(zs(g), wslice(WZX), xs, start=True, stop=True)
                cp = ps.tile([C, PG], f32, tag=f"c_{g}", bufs=2)
                nc.tensor.matmul(cp[:], wslice(WHX), xs, start=True, stop=True)
                cp_t[g] = cp

            z16 = wk.tile([C, P], f16, tag="z16")
            nc.scalar.activation(z16[:], zp[:], AF.Sigmoid, bias=bt[:, 0:1])
            h16 = [None, None]
            for g in range(G):
                ct = wk.tile([C, PG], f16, tag=f"c16_{g}")
                nc.scalar.activation(ct[:], cp_t[g][:], AF.Tanh, bias=bt[:, 2:3])
                nt = spool.tile([C, PG], f16, tag=f"h16_{g}")
                nc.vector.tensor_mul(
                    nt[:], z16[:, g * PG : (g + 1) * PG], ct[:]
                )
                h16[g] = nt
                nc.gpsimd.dma_start(o_ap[0, :, g * PG : (g + 1) * PG], nt[:])

            # openers for t=1 (Z/R free after sigmoid read; C double-buffered)
            rp_t = [None, None]
            cp_n = [None, None]
            for g in range(G):
                xs = x_next[:, g * PG : (g + 1) * PG]
                cp = ps.tile([C, PG], f32, tag=f"c_{g}", bufs=2)
                nc.tensor.matmul(cp[:], wslice(WHX), xs, start=True, stop=False)
                cp_n[g] = cp
                rp = ps.tile([C, PG], f32, tag=f"r_{g}", bufs=1)
                nc.tensor.matmul(rp[:], wslice(WRX), xs, start=True, stop=False)
                rp_t[g] = rp
                nc.tensor.matmul(zs(g), wslice(WZX), xs, start=True, stop=False)
            cp_t = cp_n

            x_t = x_next
            x_next = load_x(2)

            # -- steps 1..15 --
            for t in range(1, T):
                go = (0, 1) if t % 2 else (1, 0)

                # PE chain head: close R, then Z
                for g in go:
                    nc.tensor.matmul(
                        rp_t[g][:], wslice(WRH), h16[g][:],
                        start=False, stop=True,
                    )
                for g in go:
                    nc.tensor.matmul(
                        zs(g), wslice(WZH), h16[g][:], start=False, stop=True
                    )

                # ACT: r sigmoids first (they gate the Whh matmul)
                r16 = [None, None]
                for g in go:
                    rt = wk.tile([C, PG], f16, tag=f"r16_{g}")
                    nc.scalar.activation(
                        rt[:], rp_t[g][:], AF.Sigmoid, bias=bt[:, 1:2]
                    )
                    r16[g] = rt

                rh16 = [None, None]
                for g in go:
                    rh = wk.tile([C, PG], f16, tag=f"rh_{g}")
                    nc.vector.tensor_mul(rh[:], r16[g][:], h16[g][:])
                    rh16[g] = rh

                # ACT: z for both groups in one instruction (off-chain)
                z16 = wk.tile([C, P], f16, tag="z16")
                nc.scalar.activation(z16[:], zp[:], AF.Sigmoid, bias=bt[:, 0:1])

                for g in go:
                    nc.tensor.matmul(
                        cp_t[g][:], wslice(WHH), rh16[g][:],
                        start=False, stop=True,
                    )

                # q_g = (z-1)*h_g (off-chain).  First group's q on DVE as
                # one STT (fast path for its chain tail); second group's q
                # on GpSimd (via a small TS for z-1) to keep DVE clear.
                ga, gb = go
                q16 = [None, None]
                qa = wk.tile([C, PG], f16, tag=f"q_{ga}")
                nc.vector.scalar_tensor_tensor(
                    qa[:], z16[:, ga * PG : (ga + 1) * PG], 1.0, h16[ga][:],
                    ALU.subtract, ALU.mult,
                )
                q16[ga] = qa
                zm1 = wk.tile([C, PG], f16, tag="zm1")
                nc.vector.tensor_scalar(
                    zm1[:], z16[:, gb * PG : (gb + 1) * PG], 1.0, None,
                    ALU.subtract,
                )
                qb = wk.tile([C, PG], f16, tag=f"q_{gb}")
                nc.gpsimd.tensor_mul(qb[:], zm1[:], h16[gb][:])
                q16[gb] = qb

                # ACT: tanh per group (chain)
                c16 = [None, None]
                for g in go:
                    ct = wk.tile([C, PG], f16, tag=f"c16_{g}")
                    nc.scalar.activation(
                        ct[:], cp_t[g][:], AF.Tanh, bias=bt[:, 2:3]
                    )
                    c16[g] = ct

                # DVE chain tail: v = z*c ; h' = v - q
                for g in go:
                    vt = wk.tile([C, PG], f16, tag=f"v_{g}")
                    nc.vector.tensor_mul(
                        vt[:], z16[:, g * PG : (g + 1) * PG], c16[g][:]
                    )
                    nt = spool.tile([C, PG], f16, tag=f"h16_{g}")
                    nc.vector.tensor_sub(nt[:], vt[:], q16[g][:])
                    h16[g] = nt
                    nc.gpsimd.dma_start(
                        o_ap[t, :, g * PG : (g + 1) * PG], nt[:]
                    )

                # openers for t+1
                if t + 1 < T:
                    cp_n = [None, None]
                    for g in go:
                        xs = x_next[:, g * PG : (g + 1) * PG]
                        cp = ps.tile([C, PG], f32, tag=f"c_{g}", bufs=2)
                        nc.tensor.matmul(
                            cp[:], wslice(WHX), xs, start=True, stop=False
                        )
                        cp_n[g] = cp
                        nc.tensor.matmul(
                            rp_t[g][:], wslice(WRX), xs, start=True, stop=False
                        )
                        nc.tensor.matmul(
                            zs(g), wslice(WZX), xs, start=True, stop=False
                        )
                    cp_t = cp_n
                    x_t = x_next
                    if t + 2 < T:
                        x_next = load_x(t + 2)

    nc.compile()
    return nc


def _get_prog():
    global _PROG
    if _PROG is None:
        _PROG = _build()
    return _PROG


def _make_in_maps(video, Wz, bz, Wr, br, Wh, bh):
    w6 = np.concatenate(
        [
            Wz[:, :C].T, Wz[:, C:].T,
            Wr[:, :C].T, Wr[:, C:].T,
            Wh[:, :C].T, Wh[:, C:].T,
        ],
        axis=1,
    ).astype(np.float16)
    b3 = np.stack([bz, br, bh], axis=1).astype(np.float32)
    in_maps = []
    for core in range(NCORES):
        b_, q = divmod(core, 4)
        xs = np.ascontiguousarray(
            video[b_, :, :, q * HQ : (q + 1) * HQ, :]
        ).reshape(T, C, P).astype(np.float16)
        in_maps.append({"x_seq": xs, "wmats": w6, "biases": b3})
    return in_maps


def kernel(video, Wz, bz, Wr, br, Wh, bh):
    _ensure_paths()
    from concourse.bass_utils import run_bass_kernel_spmd

    video = np.asarray(video, dtype=np.float32)
    nc = _get_prog()
    in_maps = _make_in_maps(video, Wz, bz, Wr, br, Wh, bh)
    res = run_bass_kernel_spmd(nc, in_maps, list(range(NCORES)))

    out = np.empty((B, T, C, H, W), np.float32)
    for core in range(NCORES):
        b_, q = divmod(core, 4)
        out[b_, :, :, q * HQ : (q + 1) * HQ, :] = np.asarray(
            res.results[core]["out_seq"]
        ).astype(np.float32).reshape(T, C, HQ, W)
    return out


# revision 7
# speedup vs baseline: 1.1478x; 1.1478x over previous
"""ConvGRU Trainium2 kernel (v2).

video [B=2, T=16, C=128, H=64, W=64] f32; 1x1-conv GRU over T.
Sharding: data-parallel over (B x H/16) -> 8 cores, each core owns
P = 16*64 = 1024 pixels for all T; weights replicated.

Layout per core: channels on partitions, pixels on the free dim.
G=2 pixel groups of PG=512 form two independent recurrence chains.

Per step (h' = (1-z)h + z*tanh(c)) computed as:
    z   = sigmoid(Z + bz)            ACT [1024]   (both groups fused)
    r_g = sigmoid(R_g + br)          ACT [512]    (chain-critical, early)
    rh_g = r_g * h_g                 DVE [512]
    c_g = tanh(C_g + bh)             ACT [512]
    q   = (z - 1) * h                DVE STT [1024] (off-chain)
    v_g = z_g * c_g                  DVE [512]
    h'_g = v_g - q_g                 DVE [512]    (= z*c + (1-z)*h)
where Z/R/C are PSUM accumulations: opener (W?x @ x_{t}, issued during
step t-1) + closer (W?h @ h or rh).

Instruction-count discipline (HW-measured: mm[512]=379ns=0.417N+165,
ACT=0.833N+260, DVE TT fp16=0.521N+160): PE 12, ACT 5, DVE 8 per step.
t=0 runs a reduced step (h=0): no closes / sigr / rh / q.

Numerics: fp16 matmul inputs/gates/state, fp32 PSUM accum + fp32 bias.
"""

import os
import sys

import numpy as np

B, T, C, H, W = 2, 16, 128, 64, 64
NCORES = 8
HQ = H // 4          # 16 rows of H per core (4 H-slices x 2 batches = 8 cores)
P = HQ * W           # 1024 pixels per core
G = 2                # pixel groups per step (independent recurrence chains)
PG = P // G          # 512 pixels per group

_PROG = None


def _ensure_paths():
    for p in ("/opt/trn_rl_repo",):
        if p not in sys.path and os.path.isdir(p):
            sys.path.append(p)


def _build():
    _ensure_paths()
    import concourse.bacc as bacc
    import concourse.tile as tile
    from concourse import mybir

    f32 = mybir.dt.float32
    f16 = mybir.dt.float16
    AF = mybir.ActivationFunctionType
    ALU = mybir.AluOpType

    nc = bacc.Bacc(
        "TRN2", target_bir_lowering=False, debug=False, num_devices=NCORES
    )
    x_dram = nc.dram_tensor("x_seq", [T, C, P], f16, kind="ExternalInput")
    w_dram = nc.dram_tensor("wmats", [C, 6 * C], f16, kind="ExternalInput")
    b_dram = nc.dram_tensor("biases", [C, 3], f32, kind="ExternalInput")
    o_dram = nc.dram_tensor("out_seq", [T, C, P], f16, kind="ExternalOutput")

    x_ap = x_dram.ap()
    w_ap = w_dram.ap()
    b_ap = b_dram.ap()
    o_ap = o_dram.ap()

    WZX, WZH, WRX, WRH, WHX, WHH = range(6)

    with tile.TileContext(nc) as tc:
        with (
            tc.tile_pool(name="consts", bufs=1) as consts,
            tc.tile_pool(name="xin", bufs=3) as xpool,
            tc.tile_pool(name="state", bufs=2) as spool,
            tc.tile_pool(name="work", bufs=2) as wk,
            tc.tile_pool(name="ps", bufs=1, space="PSUM") as ps,
        ):
            # garbage tile for warmup matmuls: no DMA dependency, so the
            # PE clock ramp + ACT table load start at kernel begin and
            # fully overlap the input DMAs
            junk = consts.tile([C, PG], f16)
            nc.vector.memset(junk[:], 0.0)

            wt = consts.tile([C, 6 * C], f16)
            nc.sync.dma_start(wt[:], w_ap[:])
            bt = consts.tile([C, 3], f32)
            nc.gpsimd.dma_start(bt[:], b_ap[:])

            def wslice(i):
                return wt[:, i * C : (i + 1) * C]

            # PSUM: Z [C,1024] (2 banks), R_g [C,512] x2 (2), C_g x2 bufs=2 (4)
            zp = ps.tile([C, P], f32, tag="Z", bufs=1)

            def zs(g):
                return zp[:, g * PG : (g + 1) * PG]

            for i in range(6):
                nc.tensor.matmul(
                    zp[:, :PG], junk[:, :C], junk[:], start=True, stop=True
                )
            wtmp = wk.tile([C, PG], f16, tag="c16_0")
            nc.scalar.activation(
                wtmp[:], zp[:, :PG], AF.Sigmoid, bias=0.0
            )

            def load_x(t):
                xt = xpool.tile([C, P], f16, tag="x")
                nc.sync.dma_start(xt[:], x_ap[t])
                return xt

            x_t = load_x(0)
            x_next = load_x(1)

            # -- t = 0 (h == 0): z/c only, h1 = z*c --
            cp_t = [None, None]
            for g in range(G):
                xs = x_t[:, g * PG : (g + 1) * PG]
                nc.tensor.matmul(zs(g), wslice(WZX), xs, start=True, stop=True)
                cp = ps.tile([C, PG], f32, tag=f"c_{g}", bufs=2)
                nc.tensor.matmul(cp[:], wslice(WHX), xs, start=True, stop=True)
                cp_t[g] = cp

            z16 = wk.tile([C, P], f16, tag="z16")
            nc.scalar.activation(z16[:], zp[:], AF.Sigmoid, bias=bt[:, 0:1])
            h16 = [None, None]
            for g in range(G):
                ct = wk.tile([C, PG], f16, tag=f"c16_{g}")
                nc.scalar.activation(ct[:], cp_t[g][:], AF.Tanh, bias=bt[:, 2:3])
                nt = spool.tile([C, PG], f16, tag=f"h16_{g}")
                nc.vector.tensor_mul(
                    nt[:], z16[:, g * PG : (g + 1) * PG], ct[:]
                )
                h16[g] = nt
                nc.gpsimd.dma_start(o_ap[0, :, g * PG : (g + 1) * PG], nt[:])

            # openers for t=1 (Z/R free after sigmoid read; C double-buffered)
            rp_t = [None, None]
            cp_n = [None, None]
            for g in range(G):
                xs = x_next[:, g * PG : (g + 1) * PG]
                cp = ps.tile([C, PG], f32, tag=f"c_{g}", bufs=2)
                nc.tensor.matmul(cp[:], wslice(WHX), xs, start=True, stop=False)
                cp_n[g] = cp
                rp = ps.tile([C, PG], f32, tag=f"r_{g}", bufs=1)
                nc.tensor.matmul(rp[:], wslice(WRX), xs, start=True, stop=False)
                rp_t[g] = rp
                nc.tensor.matmul(zs(g), wslice(WZX), xs, start=True, stop=False)
            cp_t = cp_n

            x_t = x_next
            x_next = load_x(2)

            # -- steps 1..15 --
            for t in range(1, T):
                go = (0, 1) if t % 2 else (1, 0)

                # PE chain head: close R, then Z
                for g in go:
                    nc.tensor.matmul(
                        rp_t[g][:], wslice(WRH), h16[g][:],
                        start=False, stop=True,
                    )
                for g in go:
                    nc.tensor.matmul(
                        zs(g), wslice(WZH), h16[g][:], start=False, stop=True
                    )

                # ACT: r sigmoids first (they gate the Whh matmul)
                r16 = [None, None]
                for g in go:
                    rt = wk.tile([C, PG], f16, tag=f"r16_{g}")
                    nc.scalar.activation(
                        rt[:], rp_t[g][:], AF.Sigmoid, bias=bt[:, 1:2]
                    )
                    r16[g] = rt

                rh16 = [None, None]
                for g in go:
                    rh = wk.tile([C, PG], f16, tag=f"rh_{g}")
                    nc.vector.tensor_mul(rh[:], r16[g][:], h16[g][:])
                    rh16[g] = rh

                # ACT: z for both groups in one instruction (off-chain)
                z16 = wk.tile([C, P], f16, tag="z16")
                nc.scalar.activation(z16[:], zp[:], AF.Sigmoid, bias=bt[:, 0:1])

                for g in go:
                    nc.tensor.matmul(
                        cp_t[g][:], wslice(WHH), rh16[g][:],
                        start=False, stop=True,
                    )

                # ACT: tanh per group (chain)
                c16 = [None, None]
                for g in go:
                    ct = wk.tile([C, PG], f16, tag=f"c16_{g}")
                    nc.scalar.activation(
                        ct[:], cp_t[g][:], AF.Tanh, bias=bt[:, 2:3]
                    )
                    c16[g] = ct

                # DVE tail, per group in chain order: q = (z-1)*h (off-
                # chain, fills the tanh window), then v = z*c, h' = v - q.
                # Group A's tail runs before any group-B work so its chain
                # restarts ASAP.
                for g in go:
                    qt = wk.tile([C, PG], f16, tag=f"q_{g}")
                    nc.vector.scalar_tensor_tensor(
                        qt[:], z16[:, g * PG : (g + 1) * PG], 1.0, h16[g][:],
                        ALU.subtract, ALU.mult,
                    )
                    vt = wk.tile([C, PG], f16, tag=f"v_{g}")
                    nc.vector.tensor_mul(
                        vt[:], z16[:, g * PG : (g + 1) * PG], c16[g][:]
                    )
                    nt = spool.tile([C, PG], f16, tag=f"h16_{g}")
                    nc.vector.tensor_sub(nt[:], vt[:], qt[:])
                    h16[g] = nt
                    nc.gpsimd.dma_start(
                        o_ap[t, :, g * PG : (g + 1) * PG], nt[:]
                    )

                # openers for t+1
                if t + 1 < T:
                    cp_n = [None, None]
                    for g in go:
                        xs = x_next[:, g * PG : (g + 1) * PG]
                        cp = ps.tile([C, PG], f32, tag=f"c_{g}", bufs=2)
                        nc.tensor.matmul(
                            cp[:], wslice(WHX), xs, start=True, stop=False
                        )
                        cp_n[g] = cp
                        nc.tensor.matmul(
                            rp_t[g][:], wslice(WRX), xs, start=True, stop=False
                        )
                        nc.tensor.matmul(
                            zs(g), wslice(WZX), xs, start=True, stop=False
                        )
                    cp_t = cp_n
                    x_t = x_next
                    if t + 2 < T:
                        x_next = load_x(t + 2)

    nc.compile()
    return nc


def _get_prog():
    global _PROG
    if _PROG is None:
        _PROG = _build()
    return _PROG


def _make_in_maps(video, Wz, bz, Wr, br, Wh, bh):
    w6 = np.concatenate(
        [
            Wz[:, :C].T, Wz[:, C:].T,
            Wr[:, :C].T, Wr[:, C:].T,
            Wh[:, :C].T, Wh[:, C:].T,
        ],
        axis=1,
    ).astype(np.float16)
    b3 = np.stack([bz, br, bh], axis=1).astype(np.float32)
    in_maps = []
    for core in range(NCORES):
        b_, q = divmod(core, 4)
        xs = np.ascontiguousarray(
            video[b_, :, :, q * HQ : (q + 1) * HQ, :]
        ).reshape(T, C, P).astype(np.float16)
        in_maps.append({"x_seq": xs, "wmats": w6, "biases": b3})
    return in_maps


def kernel(video, Wz, bz, Wr, br, Wh, bh):
    _ensure_paths()
    from concourse.bass_utils import run_bass_kernel_spmd

    video = np.asarray(video, dtype=np.float32)
    nc = _get_prog()
    in_maps = _make_in_maps(video, Wz, bz, Wr, br, Wh, bh)
    res = run_bass_kernel_spmd(nc, in_maps, list(range(NCORES)))

    out = np.empty((B, T, C, H, W), np.float32)
    for core in range(NCORES):
        b_, q = divmod(core, 4)
        out[b_, :, :, q * HQ : (q + 1) * HQ, :] = np.asarray(
            res.results[core]["out_seq"]
        ).astype(np.float32).reshape(T, C, HQ, W)
    return out


# revision 10
# speedup vs baseline: 1.1526x; 1.0042x over previous
"""ConvGRU Trainium2 kernel (v2).

video [B=2, T=16, C=128, H=64, W=64] f32; 1x1-conv GRU over T.
Sharding: data-parallel over (B x H/16) -> 8 cores, each core owns
P = 16*64 = 1024 pixels for all T; weights replicated.

Layout per core: channels on partitions, pixels on the free dim.
G=2 pixel groups of PG=512 form two independent recurrence chains.

Per step (h' = (1-z)h + z*tanh(c)) computed as:
    z   = sigmoid(Z + bz)            ACT [1024]   (both groups fused)
    r_g = sigmoid(R_g + br)          ACT [512]    (chain-critical, early)
    rh_g = r_g * h_g                 DVE [512]
    c_g = tanh(C_g + bh)             ACT [512]
    q   = (z - 1) * h                DVE STT [1024] (off-chain)
    v_g = z_g * c_g                  DVE [512]
    h'_g = v_g - q_g                 DVE [512]    (= z*c + (1-z)*h)
where Z/R/C are PSUM accumulations: opener (W?x @ x_{t}, issued during
step t-1) + closer (W?h @ h or rh).

Instruction-count discipline (HW-measured: mm[512]=379ns=0.417N+165,
ACT=0.833N+260, DVE TT fp16=0.521N+160): PE 12, ACT 5, DVE 8 per step.
t=0 runs a reduced step (h=0): no closes / sigr / rh / q.

Numerics: fp16 matmul inputs/gates/state, fp32 PSUM accum + fp32 bias.
"""

import os
import sys

import numpy as np

B, T, C, H, W = 2, 16, 128, 64, 64
NCORES = 8
HQ = H // 4          # 16 rows of H per core (4 H-slices x 2 batches = 8 cores)
P = HQ * W           # 1024 pixels per core
G = 2                # pixel groups per step (independent recurrence chains)
PG = P // G          # 512 pixels per group

_PROG = None


def _ensure_paths():
    for p in ("/opt/trn_rl_repo",):
        if p not in sys.path and os.path.isdir(p):
            sys.path.append(p)


def _build():
    _ensure_paths()
    import concourse.bacc as bacc
    import concourse.tile as tile
    from concourse import mybir

    f32 = mybir.dt.float32
    f16 = mybir.dt.float16
    AF = mybir.ActivationFunctionType
    ALU = mybir.AluOpType

    nc = bacc.Bacc(
        "TRN2", target_bir_lowering=False, debug=False, num_devices=NCORES
    )
    x_dram = nc.dram_tensor("x_seq", [T, C, P], f16, kind="ExternalInput")
    w_dram = nc.dram_tensor("wmats", [C, 6 * C], f16, kind="ExternalInput")
    b_dram = nc.dram_tensor("biases", [C, 3], f32, kind="ExternalInput")
    o_dram = nc.dram_tensor("out_seq", [T, C, P], f16, kind="ExternalOutput")

    x_ap = x_dram.ap()
    w_ap = w_dram.ap()
    b_ap = b_dram.ap()
    o_ap = o_dram.ap()

    WZX, WZH, WRX, WRH, WHX, WHH = range(6)

    with tile.TileContext(nc) as tc:
        with (
            tc.tile_pool(name="consts", bufs=1) as consts,
            tc.tile_pool(name="xin", bufs=3) as xpool,
            tc.tile_pool(name="state", bufs=2) as spool,
            tc.tile_pool(name="work", bufs=2) as wk,
            tc.tile_pool(name="ps", bufs=1, space="PSUM") as ps,
        ):
            # garbage tile for warmup matmuls: no DMA dependency, so the
            # PE clock ramp + ACT table load start at kernel begin and
            # fully overlap the input DMAs
            junk = consts.tile([C, PG], f16)
            nc.vector.memset(junk[:], 0.0)

            wt = consts.tile([C, 6 * C], f16)
            nc.scalar.dma_start(wt[:], w_ap[:])
            bt = consts.tile([C, 3], f32)
            nc.gpsimd.dma_start(bt[:], b_ap[:])

            def wslice(i):
                return wt[:, i * C : (i + 1) * C]

            # PSUM: Z [C,1024] (2 banks), R_g [C,512] x2 (2), C_g x2 bufs=2 (4)
            zp = ps.tile([C, P], f32, tag="Z", bufs=1)

            def zs(g):
                return zp[:, g * PG : (g + 1) * PG]

            for i in range(3):
                nc.tensor.matmul(
                    zp[:, :PG], junk[:, :C], junk[:], start=True, stop=True
                )
            wtmp = wk.tile([C, PG], f16, tag="c16_0")
            nc.scalar.activation(
                wtmp[:], zp[:, :PG], AF.Sigmoid, bias=0.0
            )

            def load_x(t):
                xt = xpool.tile([C, P], f16, tag="x")
                nc.sync.dma_start(xt[:], x_ap[t])
                return xt

            x_t = load_x(0)
            x_next = load_x(1)

            # -- t = 0 (h == 0): z/c only, h1 = z*c --
            cp_t = [None, None]
            for g in range(G):
                xs = x_t[:, g * PG : (g + 1) * PG]
                nc.tensor.matmul(zs(g), wslice(WZX), xs, start=True, stop=True)
                cp = ps.tile([C, PG], f32, tag=f"c_{g}", bufs=2)
                nc.tensor.matmul(cp[:], wslice(WHX), xs, start=True, stop=True)
                cp_t[g] = cp

            z16 = wk.tile([C, P], f16, tag="z16")
            nc.scalar.activation(z16[:], zp[:], AF.Sigmoid, bias=bt[:, 0:1])
            h16 = [None, None]
            for g in range(G):
                ct = wk.tile([C, PG], f16, tag=f"c16_{g}")
                nc.scalar.activation(ct[:], cp_t[g][:], AF.Tanh, bias=bt[:, 2:3])
                nt = spool.tile([C, PG], f16, tag=f"h16_{g}")
                nc.vector.tensor_mul(
                    nt[:], z16[:, g * PG : (g + 1) * PG], ct[:]
                )
                h16[g] = nt
                nc.gpsimd.dma_start(o_ap[0, :, g * PG : (g + 1) * PG], nt[:])

            # openers for t=1 (Z/R free after sigmoid read; C double-buffered)
            rp_t = [None, None]
            cp_n = [None, None]
            for g in range(G):
                xs = x_next[:, g * PG : (g + 1) * PG]
                cp = ps.tile([C, PG], f32, tag=f"c_{g}", bufs=2)
                nc.tensor.matmul(cp[:], wslice(WHX), xs, start=True, stop=False)
                cp_n[g] = cp
                rp = ps.tile([C, PG], f32, tag=f"r_{g}", bufs=1)
                nc.tensor.matmul(rp[:], wslice(WRX), xs, start=True, stop=False)
                rp_t[g] = rp
                nc.tensor.matmul(zs(g), wslice(WZX), xs, start=True, stop=False)
            cp_t = cp_n

            x_t = x_next
            x_next = load_x(2)

            # -- steps 1..15 --
            for t in range(1, T):
                go = (0, 1) if t % 2 else (1, 0)

                # PE chain head: close R, then Z
                for g in go:
                    nc.tensor.matmul(
                        rp_t[g][:], wslice(WRH), h16[g][:],
                        start=False, stop=True,
                    )
                for g in go:
                    nc.tensor.matmul(
                        zs(g), wslice(WZH), h16[g][:], start=False, stop=True
                    )

                # ACT: r sigmoids first (they gate the Whh matmul)
                r16 = [None, None]
                for g in go:
                    rt = wk.tile([C, PG], f16, tag=f"r16_{g}")
                    nc.scalar.activation(
                        rt[:], rp_t[g][:], AF.Sigmoid, bias=bt[:, 1:2]
                    )
                    r16[g] = rt

                rh16 = [None, None]
                for g in go:
                    rh = wk.tile([C, PG], f16, tag=f"rh_{g}")
                    nc.vector.tensor_mul(rh[:], r16[g][:], h16[g][:])
                    rh16[g] = rh

                # ACT: z for both groups in one instruction (off-chain)
                z16 = wk.tile([C, P], f16, tag="z16")
                nc.scalar.activation(z16[:], zp[:], AF.Sigmoid, bias=bt[:, 0:1])

                for g in go:
                    nc.tensor.matmul(
                        cp_t[g][:], wslice(WHH), rh16[g][:],
                        start=False, stop=True,
                    )

                # ACT: tanh per group (chain)
                c16 = [None, None]
                for g in go:
                    ct = wk.tile([C, PG], f16, tag=f"c16_{g}")
                    nc.scalar.activation(
                        ct[:], cp_t[g][:], AF.Tanh, bias=bt[:, 2:3]
                    )
                    c16[g] = ct

                # DVE off-chain: zb = 1-z (one 4x-mode TS over both
                # groups), then qn_g = zb*h_g (cheap 2x TTs) fill the tanh
                # window.  Chain tail per group is only v = z*c and
                # h' = v + qn.
                zb16 = wk.tile([C, P], f16, tag="zb16")
                nc.vector.tensor_scalar(
                    zb16[:], z16[:], -1.0, 1.0, ALU.mult, ALU.add
                )
                qn16 = [None, None]
                for g in go:
                    qt = wk.tile([C, PG], f16, tag=f"q_{g}")
                    nc.vector.tensor_mul(
                        qt[:], zb16[:, g * PG : (g + 1) * PG], h16[g][:]
                    )
                    qn16[g] = qt
                for g in go:
                    vt = wk.tile([C, PG], f16, tag=f"v_{g}")
                    nc.vector.tensor_mul(
                        vt[:], z16[:, g * PG : (g + 1) * PG], c16[g][:]
                    )
                    nt = spool.tile([C, PG], f16, tag=f"h16_{g}")
                    nc.vector.tensor_add(nt[:], vt[:], qn16[g][:])
                    h16[g] = nt
                    nc.gpsimd.dma_start(
                        o_ap[t, :, g * PG : (g + 1) * PG], nt[:]
                    )

                # openers for t+1
                if t + 1 < T:
                    cp_n = [None, None]
                    for g in go:
                        xs = x_next[:, g * PG : (g + 1) * PG]
                        cp = ps.tile([C, PG], f32, tag=f"c_{g}", bufs=2)
                        nc.tensor.matmul(
                            cp[:], wslice(WHX), xs, start=True, stop=False
                        )
                        cp_n[g] = cp
                        nc.tensor.matmul(
                            rp_t[g][:], wslice(WRX), xs, start=True, stop=False
                        )
                        nc.tensor.matmul(
                            zs(g), wslice(WZX), xs, start=True, stop=False
                        )
                    cp_t = cp_n
                    x_t = x_next
                    if t + 2 < T:
                        x_next = load_x(t + 2)

    nc.compile()
    return nc


def _get_prog():
    global _PROG
    if _PROG is None:
        _PROG = _build()
    return _PROG


def _make_in_maps(video, Wz, bz, Wr, br, Wh, bh):
    w6 = np.concatenate(
        [
            Wz[:, :C].T, Wz[:, C:].T,
            Wr[:, :C].T, Wr[:, C:].T,
            Wh[:, :C].T, Wh[:, C:].T,
        ],
        axis=1,
    ).astype(np.float16)
    b3 = np.stack([bz, br, bh], axis=1).astype(np.float32)
    in_maps = []
    for core in range(NCORES):
        b_, q = divmod(core, 4)
        xs = np.ascontiguousarray(
            video[b_, :, :, q * HQ : (q + 1) * HQ, :]
        ).reshape(T, C, P).astype(np.float16)
        in_maps.append({"x_seq": xs, "wmats": w6, "biases": b3})
    return in_maps


def kernel(video, Wz, bz, Wr, br, Wh, bh):
    _ensure_paths()
    from concourse.bass_utils import run_bass_kernel_spmd

    video = np.asarray(video, dtype=np.float32)
    nc = _get_prog()
    in_maps = _make_in_maps(video, Wz, bz, Wr, br, Wh, bh)
    res = run_bass_kernel_spmd(nc, in_maps, list(range(NCORES)))

    out = np.empty((B, T, C, H, W), np.float32)
    for core in range(NCORES):
        b_, q = divmod(core, 4)
        out[b_, :, :, q * HQ : (q + 1) * HQ, :] = np.asarray(
            res.results[core]["out_seq"]
        ).astype(np.float32).reshape(T, C, HQ, W)
    return out
